# revision 23
# baseline (speedup 1.0000x reference)
"""GCN block (GCNConv + BatchNorm + ReLU) on 8 Trainium2 NeuronCores.

Strategy (graph/data parallel, per the sharding hint):
  - Shard target nodes across the 8 cores (12544 = 98 blocks x 128 targets
    per core; core 7's tail rows are padding and trimmed on the host).
  - Host precomputes symmetric-normalization coefficients and buckets
    edges (incl. self-loops) by target block, padded to whole 128-edge
    chunks, K chunks per block (uniform so one SPMD program serves all
    cores).
  - Device: for each block, a batched indirect DMA gathers the K*128
    source rows of x; for each chunk the DVE builds a one-hot selector
    S[e, t] = norm_e * (tloc_e == t); the PE accumulates
    aggT[d, t] += M_chunk.T @ S_chunk in PSUM (this is the scatter-add);
    a second matmul with the replicated 128x128 weight gives
    Y.T[dout, t] = W.T @ aggT. BN batch statistics (sum, sum-of-squares)
    are accumulated for free via the ACT engine's accum_out, all-reduced
    across the 8 cores (tiny 128x2 collective), and the final
    relu(a*Y + c) is applied as a single fused ACT op per block, then
    PE-transposed back to [node, feature] layout and DMA'd out.

  Note: the pre-BN bias b is mathematically absorbed by BatchNorm
  (shifting by a constant changes mean by the same constant), so it is
  ignored.
"""

import numpy as np

import concourse.bacc as bacc
import concourse.bass as bass
import concourse.mybir as mybir
import concourse.tile as tile
from concourse.bass import IndirectOffsetOnAxis
from concourse.bass_utils import run_bass_kernel_spmd
from concourse.masks import make_identity

N_NODES = 100000
HIDDEN = 128
N_CORES = 8
BLOCKS = 98               # target blocks per core
NSH = BLOCKS * 128        # 12544 targets per core (>= ceil(N/8))
BN_EPS = 1e-5

F32 = mybir.dt.float32
I32 = mybir.dt.int32

_compiled = {}
LAST_RESULTS = None
_K_last = None
_in_maps_last = None


def _build_program(K: int):
    """Build the SPMD Bass program for K chunks (of 128 edges) per block.

    All DVE-consumed metadata (tloc, norm, iota, gamma, beta) is packed
    into a single host-built "meta" tensor loaded by ONE dma, so DVE
    tensor_scalar instructions carry at most 2 sync waits (the
    TensorScalarPtr encoding has a very small wait budget).
      meta[:, 0:C]        = tloc  (local target index per edge, as f32)
      meta[:, C:2C]       = norm  (normalization coefficient per edge)
      meta[:, 2C:2C+128]  = iota  (row 0..127 in every partition)
      meta[:, 2C+128]     = gamma
      meta[:, 2C+129]     = beta
      meta[:, 2C+130+b]   = dinv^2 of block b's targets (self-loop coeffs)

    Self-loops are NOT in the edge list: their contribution
    dinv(c)^2 * x[c] is added via a linear read of the core's x shard and
    a matmul with a diagonal matrix diag(dinv^2) built by affine_select —
    saving ~10% of the (bottleneck) indirect gather calls.
    """
    C = BLOCKS * K
    MW = 2 * C + 130 + BLOCKS
    nc = bacc.Bacc("TRN2", num_devices=N_CORES)

    x_d = nc.dram_tensor("x", [N_NODES, HIDDEN], F32, kind="ExternalInput")
    xsh_d = nc.dram_tensor("xsh", [NSH, HIDDEN], F32, kind="ExternalInput")
    w_d = nc.dram_tensor("w_in", [HIDDEN, HIDDEN], F32, kind="ExternalInput")
    meta_d = nc.dram_tensor("meta", [128, MW], F32, kind="ExternalInput")
    srcs_d = nc.dram_tensor("srcs", [128, C], I32, kind="ExternalInput")
    out_d = nc.dram_tensor("out", [NSH, HIDDEN], F32, kind="ExternalOutput")

    with tile.TileContext(nc) as tc:
        with (
            tc.tile_pool(name="const", bufs=1) as cpool,
            tc.tile_pool(name="yres", bufs=1) as ypool,
            tc.tile_pool(name="mblk", bufs=3) as mpool,
            tc.tile_pool(name="sblk", bufs=6) as spool,
            tc.tile_pool(name="evac", bufs=3) as epool,
            tc.tile_pool(name="outp", bufs=3) as opool,
            tc.tile_pool(name="xsh", bufs=3) as xpool,
            tc.tile_pool(name="diag", bufs=3) as dgpool,
            tc.tile_pool(name="psA", bufs=2, space="PSUM") as psA,
            tc.tile_pool(name="psY", bufs=2, space="PSUM") as psY,
            tc.tile_pool(name="psT", bufs=2, space="PSUM") as psT,
            tc.tile_pool(name="dram", bufs=1, space="DRAM") as dpool,
        ):
            # ---- constants / inputs staged to SBUF ----
            w_sb = cpool.tile([128, 128], F32)
            nc.sync.dma_start(out=w_sb[:], in_=w_d[:, :])
            meta_sb = cpool.tile([128, MW], F32)
            nc.sync.dma_start(out=meta_sb[:], in_=meta_d[:, :])
            srcs_sb = cpool.tile([128, C], I32)
            nc.sync.dma_start(out=srcs_sb[:], in_=srcs_d[:, :])

            ident = cpool.tile([128, 128], F32)
            make_identity(nc, ident[:])

            y_all = ypool.tile([128, NSH], F32)
            sum_cols = cpool.tile([128, BLOCKS], F32)
            sumsq_cols = cpool.tile([128, BLOCKS], F32)

            # ---- main loop: aggregate + transform per target block ----
            for blk in range(BLOCKS):
                m_blk = mpool.tile([128, K * 128], F32, tag="m")
                # HW indirect DMA consumes exactly one index per partition
                # (128 rows per call), so gather chunk-by-chunk.
                for j in range(K):
                    c = blk * K + j
                    nc.gpsimd.indirect_dma_start(
                        out=m_blk[:, j * 128:(j + 1) * 128],
                        out_offset=None,
                        in_=x_d[:, :],
                        in_offset=IndirectOffsetOnAxis(
                            ap=srcs_sb[:, c:c + 1], axis=0
                        ),
                    )
                agg_ps = psA.tile([128, 128], F32, tag="agg", space="PSUM")
                for j in range(K):
                    c = blk * K + j
                    s_t = spool.tile([128, 128], F32, tag="s")
                    nc.vector.tensor_scalar(
                        out=s_t[:],
                        in0=meta_sb[:, 2 * C:2 * C + 128],
                        scalar1=meta_sb[:, c:c + 1],
                        scalar2=meta_sb[:, C + c:C + c + 1],
                        op0=mybir.AluOpType.is_equal,
                        op1=mybir.AluOpType.mult,
                    )
                    nc.tensor.matmul(
                        agg_ps[:],
                        lhsT=m_blk[:, j * 128:(j + 1) * 128],
                        rhs=s_t[:],
                        start=(j == 0),
                        stop=False,
                    )
                # self-loop term: accumulate (x_blk.T @ diag(dinv^2)) into
                # the same PSUM group.
                xt = xpool.tile([128, 128], F32, tag="xt")
                nc.sync.dma_start(out=xt[:],
                                  in_=xsh_d[blk * 128:(blk + 1) * 128, :])
                dg = dgpool.tile([128, 128], F32, tag="dg")
                nc.vector.tensor_scalar(
                    out=dg[:], in0=ident[:],
                    scalar1=meta_sb[:, 2 * C + 130 + blk:2 * C + 131 + blk],
                    scalar2=None, op0=mybir.AluOpType.mult,
                )
                nc.tensor.matmul(agg_ps[:], lhsT=xt[:], rhs=dg[:],
                                 start=False, stop=True)
                agg_sb = epool.tile([128, 128], F32, tag="aggsb")
                nc.scalar.copy(agg_sb[:], agg_ps[:])
                y_ps = psY.tile([128, 128], F32, tag="y", space="PSUM")
                nc.tensor.matmul(y_ps[:], lhsT=w_sb[:], rhs=agg_sb[:],
                                 start=True, stop=True)
                ysl = y_all[:, blk * 128:(blk + 1) * 128]
                nc.scalar.activation(
                    out=ysl, in_=y_ps[:],
                    func=mybir.ActivationFunctionType.Identity,
                    accum_out=sum_cols[:, blk:blk + 1],
                )
                sq_t = epool.tile([128, 128], F32, tag="sq")
                nc.scalar.activation(
                    out=sq_t[:], in_=ysl,
                    func=mybir.ActivationFunctionType.Square,
                    accum_out=sumsq_cols[:, blk:blk + 1],
                )

            # ---- global BN statistics (tiny all-reduce) ----
            stats2 = cpool.tile([128, 2], F32)
            nc.vector.tensor_reduce(stats2[:, 0:1], sum_cols[:],
                                    axis=mybir.AxisListType.X,
                                    op=mybir.AluOpType.add)
            nc.vector.tensor_reduce(stats2[:, 1:2], sumsq_cols[:],
                                    axis=mybir.AxisListType.X,
                                    op=mybir.AluOpType.add)
            cc_in = dpool.tile([128, 2], F32)
            cc_out = dpool.tile([128, 2], F32, addr_space="Shared")
            nc.sync.dma_start(out=cc_in[:], in_=stats2[:])
            nc.gpsimd.collective_compute(
                "AllReduce",
                mybir.AluOpType.add,
                replica_groups=[list(range(N_CORES))],
                ins=[cc_in.opt()],
                outs=[cc_out.opt()],
            )
            gst = cpool.tile([128, 2], F32)
            nc.sync.dma_start(out=gst[:], in_=cc_out[:])

            inv_n = 1.0 / float(N_NODES)
            mean = cpool.tile([128, 1], F32)
            nc.vector.tensor_scalar(out=mean[:], in0=gst[:, 0:1],
                                    scalar1=inv_n, scalar2=None,
                                    op0=mybir.AluOpType.mult)
            ex2 = cpool.tile([128, 1], F32)
            nc.vector.tensor_scalar(out=ex2[:], in0=gst[:, 1:2],
                                    scalar1=inv_n, scalar2=None,
                                    op0=mybir.AluOpType.mult)
            mean2 = cpool.tile([128, 1], F32)
            nc.vector.tensor_tensor(out=mean2[:], in0=mean[:], in1=mean[:],
                                    op=mybir.AluOpType.mult)
            var = cpool.tile([128, 1], F32)
            nc.vector.tensor_tensor(out=var[:], in0=ex2[:], in1=mean2[:],
                                    op=mybir.AluOpType.subtract)
            eps_t = cpool.tile([128, 1], F32)
            nc.vector.memset(eps_t[:], float(BN_EPS))
            sdv = cpool.tile([128, 1], F32)
            nc.scalar.activation(out=sdv[:], in_=var[:],
                                 func=mybir.ActivationFunctionType.Sqrt,
                                 bias=eps_t[:])
            inv_std = cpool.tile([128, 1], F32)
            nc.vector.reciprocal(inv_std[:], sdv[:])
            a_col = cpool.tile([128, 1], F32)
            nc.vector.tensor_tensor(out=a_col[:],
                                    in0=meta_sb[:, 2 * C + 128:2 * C + 129],
                                    in1=inv_std[:], op=mybir.AluOpType.mult)
            ma = cpool.tile([128, 1], F32)
            nc.vector.tensor_tensor(out=ma[:], in0=mean[:], in1=a_col[:],
                                    op=mybir.AluOpType.mult)
            c_col = cpool.tile([128, 1], F32)
            nc.vector.tensor_tensor(out=c_col[:],
                                    in0=meta_sb[:, 2 * C + 129:2 * C + 130],
                                    in1=ma[:], op=mybir.AluOpType.subtract)

            # ---- apply BN + ReLU, transpose back, write out ----
            for blk in range(BLOCKS):
                yn = opool.tile([128, 128], F32, tag="yn")
                nc.scalar.activation(
                    out=yn[:], in_=y_all[:, blk * 128:(blk + 1) * 128],
                    func=mybir.ActivationFunctionType.Relu,
                    bias=c_col[:], scale=a_col[:],
                )
                t_ps = psT.tile([128, 128], F32, tag="t", space="PSUM")
                nc.tensor.transpose(t_ps[:], yn[:], ident[:])
                osb = opool.tile([128, 128], F32, tag="osb")
                nc.vector.tensor_copy(osb[:], t_ps[:])
                nc.sync.dma_start(out=out_d[blk * 128:(blk + 1) * 128, :],
                                  in_=osb[:])
    nc.finalize()
    return nc


def _preprocess(edge_index):
    """Bucket edges by target into per-core chunk arrays.

    Self-loops are handled separately (diag term), but the degree counts
    include them, matching the reference.
    """
    row = np.asarray(edge_index[0], dtype=np.int64)
    col = np.asarray(edge_index[1], dtype=np.int64)

    deg = (np.bincount(col, minlength=N_NODES) + 1).astype(np.float32)
    dinv = 1.0 / np.sqrt(np.maximum(deg, 1.0))
    norm = (dinv[row] * dinv[col]).astype(np.float32)
    dinv2 = (dinv * dinv).astype(np.float32)

    order = np.argsort(col, kind="stable")
    row_s = row[order]
    col_s = col[order]
    norm_s = norm[order]

    n_blocks_tot = N_CORES * BLOCKS
    b = (col_s >> 7).astype(np.int64)          # global 128-target block id
    cnt = np.bincount(b, minlength=n_blocks_tot)
    K = int(np.ceil(cnt.max() / 128.0))
    K = max(K, 1)
    C = BLOCKS * K

    starts = np.concatenate([[0], np.cumsum(cnt)])[:-1]
    r = np.arange(len(col_s)) - starts[b]       # rank within block
    chunk_in_blk = r // 128
    lane = r % 128
    core = b // BLOCKS
    blk_local = b % BLOCKS
    chunk_col = blk_local * K + chunk_in_blk

    srcs = np.zeros((N_CORES, C, 128), dtype=np.int32)
    tloc = np.zeros((N_CORES, C, 128), dtype=np.float32)
    nrm = np.zeros((N_CORES, C, 128), dtype=np.float32)
    srcs[core, chunk_col, lane] = row_s.astype(np.int32)
    tloc[core, chunk_col, lane] = (col_s & 127).astype(np.float32)
    nrm[core, chunk_col, lane] = norm_s

    # device wants [128 partitions, C chunks]
    srcs = np.ascontiguousarray(srcs.transpose(0, 2, 1))
    tloc = np.ascontiguousarray(tloc.transpose(0, 2, 1))
    nrm = np.ascontiguousarray(nrm.transpose(0, 2, 1))

    # dinv^2 per (core, block-lane): [8, 128, BLOCKS]
    d2 = np.zeros(N_CORES * NSH, dtype=np.float32)
    d2[:N_NODES] = dinv2
    d2 = d2.reshape(N_CORES, BLOCKS, 128).transpose(0, 2, 1)
    d2 = np.ascontiguousarray(d2)
    return K, srcs, tloc, nrm, d2


def _build_meta(K, tloc, nrm, d2, gamma, beta):
    """Pack tloc | norm | iota | gamma | beta | dinv2 into one meta array."""
    C = BLOCKS * K
    meta = np.zeros((N_CORES, 128, 2 * C + 130 + BLOCKS), dtype=np.float32)
    iota = np.tile(np.arange(128, dtype=np.float32)[None, :], (128, 1))
    for k in range(N_CORES):
        meta[k, :, 0:C] = tloc[k]
        meta[k, :, C:2 * C] = nrm[k]
        meta[k, :, 2 * C:2 * C + 128] = iota
        meta[k, :, 2 * C + 128] = gamma
        meta[k, :, 2 * C + 129] = beta
        meta[k, :, 2 * C + 130:] = d2[k]
    return meta


def kernel(x, edge_index, W, b, gamma, beta, _trace=False):
    global LAST_RESULTS
    x = np.ascontiguousarray(np.asarray(x, dtype=np.float32))
    W = np.ascontiguousarray(np.asarray(W, dtype=np.float32))
    gamma = np.ascontiguousarray(np.asarray(gamma, dtype=np.float32))
    beta = np.ascontiguousarray(np.asarray(beta, dtype=np.float32))

    K, srcs, tloc, nrm, d2 = _preprocess(np.asarray(edge_index))
    meta = _build_meta(K, tloc, nrm, d2, gamma, beta)

    xsh = np.zeros((N_CORES * NSH, HIDDEN), dtype=np.float32)
    xsh[:N_NODES] = x
    xsh = xsh.reshape(N_CORES, NSH, HIDDEN)

    if K not in _compiled:
        _compiled[K] = _build_program(K)
    nc = _compiled[K]

    in_maps = []
    for k in range(N_CORES):
        in_maps.append({
            "x": x,
            "xsh": np.ascontiguousarray(xsh[k]),
            "w_in": W,
            "meta": meta[k],
            "srcs": srcs[k],
        })
    global _K_last, _in_maps_last
    _K_last = K
    _in_maps_last = in_maps
    res = run_bass_kernel_spmd(nc, in_maps, core_ids=list(range(N_CORES)),
                               trace=_trace)
    LAST_RESULTS = res
    outs = [res.results[k]["out"] for k in range(N_CORES)]
    full = np.concatenate(outs, axis=0)[:N_NODES]
    return np.ascontiguousarray(full.astype(np.float32))


# revision 25
# speedup vs baseline: 1.4668x; 1.4668x over previous
"""GCN block (GCNConv + BatchNorm + ReLU) on 8 Trainium2 NeuronCores.

Strategy (graph/data parallel, per the sharding hint):
  - Shard target nodes across the 8 cores (12544 = 98 blocks x 128 targets
    per core; core 7's tail rows are padding and trimmed on the host).
  - Host precomputes symmetric-normalization coefficients and buckets
    edges (incl. self-loops) by target block, padded to whole 128-edge
    chunks, K chunks per block (uniform so one SPMD program serves all
    cores).
  - Device: for each block, a batched indirect DMA gathers the K*128
    source rows of x; for each chunk the DVE builds a one-hot selector
    S[e, t] = norm_e * (tloc_e == t); the PE accumulates
    aggT[d, t] += M_chunk.T @ S_chunk in PSUM (this is the scatter-add);
    a second matmul with the replicated 128x128 weight gives
    Y.T[dout, t] = W.T @ aggT. BN batch statistics (sum, sum-of-squares)
    are accumulated for free via the ACT engine's accum_out, all-reduced
    across the 8 cores (tiny 128x2 collective), and the final
    relu(a*Y + c) is applied as a single fused ACT op per block, then
    PE-transposed back to [node, feature] layout and DMA'd out.

  Note: the pre-BN bias b is mathematically absorbed by BatchNorm
  (shifting by a constant changes mean by the same constant), so it is
  ignored.
"""

import numpy as np

import concourse.bacc as bacc
import concourse.bass as bass
import concourse.mybir as mybir
import concourse.tile as tile
from concourse.bass import IndirectOffsetOnAxis
from concourse.bass_utils import run_bass_kernel_spmd
from concourse.masks import make_identity

N_NODES = 100000
HIDDEN = 128
N_CORES = 8
BLOCKS = 98               # target blocks per core
NSH = BLOCKS * 128        # 12544 targets per core (>= ceil(N/8))
BN_EPS = 1e-5

F32 = mybir.dt.float32
I32 = mybir.dt.int32

_compiled = {}
LAST_RESULTS = None
_K_last = None
_in_maps_last = None


def _build_program(K: int, reps: int = 1):
    """Build the SPMD Bass program for K chunks (of 128 edges) per block.

    reps > 1 statically repeats the main aggregation loop (identical work)
    for timing purposes only — the (R=3) - (R=1) wall-clock delta isolates
    the main-loop time from the ~90 ms axon dispatch noise.

    All DVE-consumed metadata (tloc, norm, iota, gamma, beta) is packed
    into a single host-built "meta" tensor loaded by ONE dma, so DVE
    tensor_scalar instructions carry at most 2 sync waits (the
    TensorScalarPtr encoding has a very small wait budget).
      meta[:, 0:C]        = tloc  (local target index per edge, as f32)
      meta[:, C:2C]       = norm  (normalization coefficient per edge)
      meta[:, 2C:2C+128]  = iota  (row 0..127 in every partition)
      meta[:, 2C+128]     = gamma
      meta[:, 2C+129]     = beta
      meta[:, 2C+130+b]   = dinv^2 of block b's targets (self-loop coeffs)

    Self-loops are NOT in the edge list: their contribution
    dinv(c)^2 * x[c] is added via a linear read of the core's x shard and
    a matmul with a diagonal matrix diag(dinv^2) built by affine_select —
    saving ~10% of the (bottleneck) indirect gather calls.
    """
    C = BLOCKS * K
    MW = 2 * C + 130 + BLOCKS
    nc = bacc.Bacc("TRN2", num_devices=N_CORES)

    x_d = nc.dram_tensor("x", [N_NODES, HIDDEN], F32, kind="ExternalInput")
    xsh_d = nc.dram_tensor("xsh", [NSH, HIDDEN], F32, kind="ExternalInput")
    w_d = nc.dram_tensor("w_in", [HIDDEN, HIDDEN], F32, kind="ExternalInput")
    meta_d = nc.dram_tensor("meta", [128, MW], F32, kind="ExternalInput")
    srcs_d = nc.dram_tensor("srcs", [128, C], I32, kind="ExternalInput")
    out_d = nc.dram_tensor("out", [NSH, HIDDEN], F32, kind="ExternalOutput")

    with tile.TileContext(nc) as tc:
        with (
            tc.tile_pool(name="const", bufs=1) as cpool,
            tc.tile_pool(name="yres", bufs=1) as ypool,
            tc.tile_pool(name="mblk", bufs=3) as mpool,
            tc.tile_pool(name="sblk", bufs=6) as spool,
            tc.tile_pool(name="evac", bufs=3) as epool,
            tc.tile_pool(name="outp", bufs=3) as opool,
            tc.tile_pool(name="xsh", bufs=3) as xpool,
            tc.tile_pool(name="diag", bufs=3) as dgpool,
            tc.tile_pool(name="psA", bufs=2, space="PSUM") as psA,
            tc.tile_pool(name="psY", bufs=2, space="PSUM") as psY,
            tc.tile_pool(name="psT", bufs=2, space="PSUM") as psT,
            tc.tile_pool(name="dram", bufs=1, space="DRAM") as dpool,
        ):
            # ---- constants / inputs staged to SBUF ----
            w_sb = cpool.tile([128, 128], F32)
            nc.sync.dma_start(out=w_sb[:], in_=w_d[:, :])
            meta_sb = cpool.tile([128, MW], F32)
            nc.sync.dma_start(out=meta_sb[:], in_=meta_d[:, :])
            srcs_sb = cpool.tile([128, C], I32)
            nc.sync.dma_start(out=srcs_sb[:], in_=srcs_d[:, :])

            ident = cpool.tile([128, 128], F32)
            make_identity(nc, ident[:])

            y_all = ypool.tile([128, NSH], F32)
            sum_cols = cpool.tile([128, BLOCKS], F32)
            sumsq_cols = cpool.tile([128, BLOCKS], F32)

            # ---- main loop: aggregate + transform per target block ----
            for _rep in range(reps):
              for blk in range(BLOCKS):
                m_blk = mpool.tile([128, K * 128], F32, tag="m")
                # HW indirect DMA consumes exactly one index per partition
                # (128 rows per call), so gather chunk-by-chunk.
                for j in range(K):
                    c = blk * K + j
                    nc.gpsimd.indirect_dma_start(
                        out=m_blk[:, j * 128:(j + 1) * 128],
                        out_offset=None,
                        in_=x_d[:, :],
                        in_offset=IndirectOffsetOnAxis(
                            ap=srcs_sb[:, c:c + 1], axis=0
                        ),
                    )
                agg_ps = psA.tile([128, 128], F32, tag="agg", space="PSUM")
                for j in range(K):
                    c = blk * K + j
                    s_t = spool.tile([128, 128], F32, tag="s")
                    nc.vector.tensor_scalar(
                        out=s_t[:],
                        in0=meta_sb[:, 2 * C:2 * C + 128],
                        scalar1=meta_sb[:, c:c + 1],
                        scalar2=meta_sb[:, C + c:C + c + 1],
                        op0=mybir.AluOpType.is_equal,
                        op1=mybir.AluOpType.mult,
                    )
                    nc.tensor.matmul(
                        agg_ps[:],
                        lhsT=m_blk[:, j * 128:(j + 1) * 128],
                        rhs=s_t[:],
                        start=(j == 0),
                        stop=False,
                    )
                # self-loop term: accumulate (x_blk.T @ diag(dinv^2)) into
                # the same PSUM group.
                xt = xpool.tile([128, 128], F32, tag="xt")
                nc.sync.dma_start(out=xt[:],
                                  in_=xsh_d[blk * 128:(blk + 1) * 128, :])
                dg = dgpool.tile([128, 128], F32, tag="dg")
                nc.vector.tensor_scalar(
                    out=dg[:], in0=ident[:],
                    scalar1=meta_sb[:, 2 * C + 130 + blk:2 * C + 131 + blk],
                    scalar2=None, op0=mybir.AluOpType.mult,
                )
                nc.tensor.matmul(agg_ps[:], lhsT=xt[:], rhs=dg[:],
                                 start=False, stop=True)
                agg_sb = epool.tile([128, 128], F32, tag="aggsb")
                nc.scalar.copy(agg_sb[:], agg_ps[:])
                y_ps = psY.tile([128, 128], F32, tag="y", space="PSUM")
                nc.tensor.matmul(y_ps[:], lhsT=w_sb[:], rhs=agg_sb[:],
                                 start=True, stop=True)
                ysl = y_all[:, blk * 128:(blk + 1) * 128]
                nc.scalar.activation(
                    out=ysl, in_=y_ps[:],
                    func=mybir.ActivationFunctionType.Identity,
                    accum_out=sum_cols[:, blk:blk + 1],
                )
                sq_t = epool.tile([128, 128], F32, tag="sq")
                nc.scalar.activation(
                    out=sq_t[:], in_=ysl,
                    func=mybir.ActivationFunctionType.Square,
                    accum_out=sumsq_cols[:, blk:blk + 1],
                )

            # ---- global BN statistics (tiny all-reduce) ----
            stats2 = cpool.tile([128, 2], F32)
            nc.vector.tensor_reduce(stats2[:, 0:1], sum_cols[:],
                                    axis=mybir.AxisListType.X,
                                    op=mybir.AluOpType.add)
            nc.vector.tensor_reduce(stats2[:, 1:2], sumsq_cols[:],
                                    axis=mybir.AxisListType.X,
                                    op=mybir.AluOpType.add)
            cc_in = dpool.tile([128, 2], F32)
            cc_out = dpool.tile([128, 2], F32, addr_space="Shared")
            nc.sync.dma_start(out=cc_in[:], in_=stats2[:])
            nc.gpsimd.collective_compute(
                "AllReduce",
                mybir.AluOpType.add,
                replica_groups=[list(range(N_CORES))],
                ins=[cc_in.opt()],
                outs=[cc_out.opt()],
            )
            gst = cpool.tile([128, 2], F32)
            nc.sync.dma_start(out=gst[:], in_=cc_out[:])

            inv_n = 1.0 / float(N_NODES)
            mean = cpool.tile([128, 1], F32)
            nc.vector.tensor_scalar(out=mean[:], in0=gst[:, 0:1],
                                    scalar1=inv_n, scalar2=None,
                                    op0=mybir.AluOpType.mult)
            ex2 = cpool.tile([128, 1], F32)
            nc.vector.tensor_scalar(out=ex2[:], in0=gst[:, 1:2],
                                    scalar1=inv_n, scalar2=None,
                                    op0=mybir.AluOpType.mult)
            mean2 = cpool.tile([128, 1], F32)
            nc.vector.tensor_tensor(out=mean2[:], in0=mean[:], in1=mean[:],
                                    op=mybir.AluOpType.mult)
            var = cpool.tile([128, 1], F32)
            nc.vector.tensor_tensor(out=var[:], in0=ex2[:], in1=mean2[:],
                                    op=mybir.AluOpType.subtract)
            eps_t = cpool.tile([128, 1], F32)
            nc.vector.memset(eps_t[:], float(BN_EPS))
            sdv = cpool.tile([128, 1], F32)
            nc.scalar.activation(out=sdv[:], in_=var[:],
                                 func=mybir.ActivationFunctionType.Sqrt,
                                 bias=eps_t[:])
            inv_std = cpool.tile([128, 1], F32)
            nc.vector.reciprocal(inv_std[:], sdv[:])
            a_col = cpool.tile([128, 1], F32)
            nc.vector.tensor_tensor(out=a_col[:],
                                    in0=meta_sb[:, 2 * C + 128:2 * C + 129],
                                    in1=inv_std[:], op=mybir.AluOpType.mult)
            ma = cpool.tile([128, 1], F32)
            nc.vector.tensor_tensor(out=ma[:], in0=mean[:], in1=a_col[:],
                                    op=mybir.AluOpType.mult)
            c_col = cpool.tile([128, 1], F32)
            nc.vector.tensor_tensor(out=c_col[:],
                                    in0=meta_sb[:, 2 * C + 129:2 * C + 130],
                                    in1=ma[:], op=mybir.AluOpType.subtract)

            # ---- apply BN + ReLU, transpose back, write out ----
            for blk in range(BLOCKS):
                yn = opool.tile([128, 128], F32, tag="yn")
                nc.scalar.activation(
                    out=yn[:], in_=y_all[:, blk * 128:(blk + 1) * 128],
                    func=mybir.ActivationFunctionType.Relu,
                    bias=c_col[:], scale=a_col[:],
                )
                t_ps = psT.tile([128, 128], F32, tag="t", space="PSUM")
                nc.tensor.transpose(t_ps[:], yn[:], ident[:])
                osb = opool.tile([128, 128], F32, tag="osb")
                nc.vector.tensor_copy(osb[:], t_ps[:])
                nc.sync.dma_start(out=out_d[blk * 128:(blk + 1) * 128, :],
                                  in_=osb[:])
    nc.finalize()
    return nc


def _preprocess(edge_index):
    """Bucket edges by target into per-core chunk arrays.

    Self-loops are handled separately (diag term), but the degree counts
    include them, matching the reference.
    """
    row = np.asarray(edge_index[0], dtype=np.int64)
    col = np.asarray(edge_index[1], dtype=np.int64)

    deg = (np.bincount(col, minlength=N_NODES) + 1).astype(np.float32)
    dinv = 1.0 / np.sqrt(np.maximum(deg, 1.0))
    norm = (dinv[row] * dinv[col]).astype(np.float32)
    dinv2 = (dinv * dinv).astype(np.float32)

    order = np.argsort(col, kind="stable")
    row_s = row[order]
    col_s = col[order]
    norm_s = norm[order]

    n_blocks_tot = N_CORES * BLOCKS
    b = (col_s >> 7).astype(np.int64)          # global 128-target block id
    cnt = np.bincount(b, minlength=n_blocks_tot)
    K = int(np.ceil(cnt.max() / 128.0))
    K = max(K, 1)
    C = BLOCKS * K

    starts = np.concatenate([[0], np.cumsum(cnt)])[:-1]
    r = np.arange(len(col_s)) - starts[b]       # rank within block
    chunk_in_blk = r // 128
    lane = r % 128
    core = b // BLOCKS
    blk_local = b % BLOCKS
    chunk_col = blk_local * K + chunk_in_blk

    srcs = np.zeros((N_CORES, C, 128), dtype=np.int32)
    tloc = np.zeros((N_CORES, C, 128), dtype=np.float32)
    nrm = np.zeros((N_CORES, C, 128), dtype=np.float32)
    srcs[core, chunk_col, lane] = row_s.astype(np.int32)
    tloc[core, chunk_col, lane] = (col_s & 127).astype(np.float32)
    nrm[core, chunk_col, lane] = norm_s

    # device wants [128 partitions, C chunks]
    srcs = np.ascontiguousarray(srcs.transpose(0, 2, 1))
    tloc = np.ascontiguousarray(tloc.transpose(0, 2, 1))
    nrm = np.ascontiguousarray(nrm.transpose(0, 2, 1))

    # dinv^2 per (core, block-lane): [8, 128, BLOCKS]
    d2 = np.zeros(N_CORES * NSH, dtype=np.float32)
    d2[:N_NODES] = dinv2
    d2 = d2.reshape(N_CORES, BLOCKS, 128).transpose(0, 2, 1)
    d2 = np.ascontiguousarray(d2)
    return K, srcs, tloc, nrm, d2


def _build_meta(K, tloc, nrm, d2, gamma, beta):
    """Pack tloc | norm | iota | gamma | beta | dinv2 into one meta array."""
    C = BLOCKS * K
    meta = np.zeros((N_CORES, 128, 2 * C + 130 + BLOCKS), dtype=np.float32)
    iota = np.tile(np.arange(128, dtype=np.float32)[None, :], (128, 1))
    for k in range(N_CORES):
        meta[k, :, 0:C] = tloc[k]
        meta[k, :, C:2 * C] = nrm[k]
        meta[k, :, 2 * C:2 * C + 128] = iota
        meta[k, :, 2 * C + 128] = gamma
        meta[k, :, 2 * C + 129] = beta
        meta[k, :, 2 * C + 130:] = d2[k]
    return meta


def kernel(x, edge_index, W, b, gamma, beta, _trace=False):
    global LAST_RESULTS
    x = np.ascontiguousarray(np.asarray(x, dtype=np.float32))
    W = np.ascontiguousarray(np.asarray(W, dtype=np.float32))
    gamma = np.ascontiguousarray(np.asarray(gamma, dtype=np.float32))
    beta = np.ascontiguousarray(np.asarray(beta, dtype=np.float32))

    K, srcs, tloc, nrm, d2 = _preprocess(np.asarray(edge_index))
    meta = _build_meta(K, tloc, nrm, d2, gamma, beta)

    xsh = np.zeros((N_CORES * NSH, HIDDEN), dtype=np.float32)
    xsh[:N_NODES] = x
    xsh = xsh.reshape(N_CORES, NSH, HIDDEN)

    if K not in _compiled:
        _compiled[K] = _build_program(K)
    nc = _compiled[K]

    in_maps = []
    for k in range(N_CORES):
        in_maps.append({
            "x": x,
            "xsh": np.ascontiguousarray(xsh[k]),
            "w_in": W,
            "meta": meta[k],
            "srcs": srcs[k],
        })
    global _K_last, _in_maps_last
    _K_last = K
    _in_maps_last = in_maps
    res = run_bass_kernel_spmd(nc, in_maps, core_ids=list(range(N_CORES)),
                               trace=_trace)
    LAST_RESULTS = res
    outs = [res.results[k]["out"] for k in range(N_CORES)]
    full = np.concatenate(outs, axis=0)[:N_NODES]
    return np.ascontiguousarray(full.astype(np.float32))


# revision 26
# speedup vs baseline: 1.6670x; 1.1365x over previous
"""GCN block (GCNConv + BatchNorm + ReLU) on 8 Trainium2 NeuronCores.

Strategy (graph/data parallel, per the sharding hint):
  - Shard target nodes across the 8 cores (12544 = 98 blocks x 128 targets
    per core; core 7's tail rows are padding and trimmed on the host).
  - Host precomputes symmetric-normalization coefficients and buckets
    edges (incl. self-loops) by target block, padded to whole 128-edge
    chunks, K chunks per block (uniform so one SPMD program serves all
    cores).
  - Device: for each block, a batched indirect DMA gathers the K*128
    source rows of x; for each chunk the DVE builds a one-hot selector
    S[e, t] = norm_e * (tloc_e == t); the PE accumulates
    aggT[d, t] += M_chunk.T @ S_chunk in PSUM (this is the scatter-add);
    a second matmul with the replicated 128x128 weight gives
    Y.T[dout, t] = W.T @ aggT. BN batch statistics (sum, sum-of-squares)
    are accumulated for free via the ACT engine's accum_out, all-reduced
    across the 8 cores (tiny 128x2 collective), and the final
    relu(a*Y + c) is applied as a single fused ACT op per block, then
    PE-transposed back to [node, feature] layout and DMA'd out.

  Note: the pre-BN bias b is mathematically absorbed by BatchNorm
  (shifting by a constant changes mean by the same constant), so it is
  ignored.
"""

import numpy as np

import concourse.bacc as bacc
import concourse.bass as bass
import concourse.mybir as mybir
import concourse.tile as tile
from concourse.bass import IndirectOffsetOnAxis
from concourse.bass_utils import run_bass_kernel_spmd
from concourse.masks import make_identity

N_NODES = 100000
HIDDEN = 128
N_CORES = 8
BLOCKS = 98               # target blocks per core
NSH = BLOCKS * 128        # 12544 targets per core (>= ceil(N/8))
BN_EPS = 1e-5

F32 = mybir.dt.float32
I32 = mybir.dt.int32

_compiled = {}
LAST_RESULTS = None
_K_last = None
_in_maps_last = None


def _build_program(K: int, reps: int = 1):
    """Build the SPMD Bass program for K chunks (of 128 edges) per block.

    reps > 1 statically repeats the main aggregation loop (identical work)
    for timing purposes only — the (R=3) - (R=1) wall-clock delta isolates
    the main-loop time from the ~90 ms axon dispatch noise.

    All DVE-consumed metadata (tloc, norm, iota, gamma, beta) is packed
    into a single host-built "meta" tensor loaded by ONE dma, so DVE
    tensor_scalar instructions carry at most 2 sync waits (the
    TensorScalarPtr encoding has a very small wait budget).
      meta[:, 0:C]        = tloc  (local target index per edge, as f32)
      meta[:, C:2C]       = norm  (normalization coefficient per edge)
      meta[:, 2C:2C+128]  = iota  (row 0..127 in every partition)
      meta[:, 2C+128]     = gamma
      meta[:, 2C+129]     = beta
      meta[:, 2C+130+b]   = dinv^2 of block b's targets (self-loop coeffs)

    Self-loops are NOT in the edge list: their contribution
    dinv(c)^2 * x[c] is added via a linear read of the core's x shard and
    a matmul with a diagonal matrix diag(dinv^2) built by affine_select —
    saving ~10% of the (bottleneck) indirect gather calls.
    """
    C = BLOCKS * K
    MW = 2 * C + 130 + BLOCKS
    nc = bacc.Bacc("TRN2", num_devices=N_CORES)

    x_d = nc.dram_tensor("x", [N_NODES, HIDDEN], F32, kind="ExternalInput")
    xsh_d = nc.dram_tensor("xsh", [NSH, HIDDEN], F32, kind="ExternalInput")
    w_d = nc.dram_tensor("w_in", [HIDDEN, HIDDEN], F32, kind="ExternalInput")
    meta_d = nc.dram_tensor("meta", [128, MW], F32, kind="ExternalInput")
    srcs_d = nc.dram_tensor("srcs", [128, C], I32, kind="ExternalInput")
    out_d = nc.dram_tensor("out", [NSH, HIDDEN], F32, kind="ExternalOutput")

    with tile.TileContext(nc) as tc:
        with (
            tc.tile_pool(name="const", bufs=1) as cpool,
            tc.tile_pool(name="yres", bufs=1) as ypool,
            tc.tile_pool(name="mblk", bufs=5) as mpool,
            tc.tile_pool(name="sblk", bufs=10) as spool,
            tc.tile_pool(name="evac", bufs=3) as epool,
            tc.tile_pool(name="outp", bufs=3) as opool,
            tc.tile_pool(name="xsh", bufs=3) as xpool,
            tc.tile_pool(name="diag", bufs=3) as dgpool,
            tc.tile_pool(name="psA", bufs=2, space="PSUM") as psA,
            tc.tile_pool(name="psY", bufs=2, space="PSUM") as psY,
            tc.tile_pool(name="psT", bufs=2, space="PSUM") as psT,
            tc.tile_pool(name="dram", bufs=1, space="DRAM") as dpool,
        ):
            # ---- constants / inputs staged to SBUF ----
            w_sb = cpool.tile([128, 128], F32)
            nc.sync.dma_start(out=w_sb[:], in_=w_d[:, :])
            meta_sb = cpool.tile([128, MW], F32)
            nc.sync.dma_start(out=meta_sb[:], in_=meta_d[:, :])
            srcs_sb = cpool.tile([128, C], I32)
            nc.sync.dma_start(out=srcs_sb[:], in_=srcs_d[:, :])

            ident = cpool.tile([128, 128], F32)
            make_identity(nc, ident[:])

            y_all = ypool.tile([128, NSH], F32)
            sum_cols = cpool.tile([128, BLOCKS], F32)
            sumsq_cols = cpool.tile([128, BLOCKS], F32)

            # ---- main loop: aggregate + transform per target block ----
            for _rep in range(reps):
              for blk in range(BLOCKS):
                m_blk = mpool.tile([128, K * 128], F32, tag="m")
                # HW indirect DMA consumes exactly one index per partition
                # (128 rows per call), so gather chunk-by-chunk.
                for j in range(K):
                    c = blk * K + j
                    nc.gpsimd.indirect_dma_start(
                        out=m_blk[:, j * 128:(j + 1) * 128],
                        out_offset=None,
                        in_=x_d[:, :],
                        in_offset=IndirectOffsetOnAxis(
                            ap=srcs_sb[:, c:c + 1], axis=0
                        ),
                    )
                agg_ps = psA.tile([128, 128], F32, tag="agg", space="PSUM")
                for j in range(K):
                    c = blk * K + j
                    s_t = spool.tile([128, 128], F32, tag="s")
                    nc.vector.tensor_scalar(
                        out=s_t[:],
                        in0=meta_sb[:, 2 * C:2 * C + 128],
                        scalar1=meta_sb[:, c:c + 1],
                        scalar2=meta_sb[:, C + c:C + c + 1],
                        op0=mybir.AluOpType.is_equal,
                        op1=mybir.AluOpType.mult,
                    )
                    nc.tensor.matmul(
                        agg_ps[:],
                        lhsT=m_blk[:, j * 128:(j + 1) * 128],
                        rhs=s_t[:],
                        start=(j == 0),
                        stop=False,
                    )
                # self-loop term: accumulate (x_blk.T @ diag(dinv^2)) into
                # the same PSUM group.
                xt = xpool.tile([128, 128], F32, tag="xt")
                nc.sync.dma_start(out=xt[:],
                                  in_=xsh_d[blk * 128:(blk + 1) * 128, :])
                dg = dgpool.tile([128, 128], F32, tag="dg")
                nc.vector.tensor_scalar(
                    out=dg[:], in0=ident[:],
                    scalar1=meta_sb[:, 2 * C + 130 + blk:2 * C + 131 + blk],
                    scalar2=None, op0=mybir.AluOpType.mult,
                )
                nc.tensor.matmul(agg_ps[:], lhsT=xt[:], rhs=dg[:],
                                 start=False, stop=True)
                agg_sb = epool.tile([128, 128], F32, tag="aggsb")
                nc.scalar.copy(agg_sb[:], agg_ps[:])
                y_ps = psY.tile([128, 128], F32, tag="y", space="PSUM")
                nc.tensor.matmul(y_ps[:], lhsT=w_sb[:], rhs=agg_sb[:],
                                 start=True, stop=True)
                ysl = y_all[:, blk * 128:(blk + 1) * 128]
                nc.scalar.activation(
                    out=ysl, in_=y_ps[:],
                    func=mybir.ActivationFunctionType.Identity,
                    accum_out=sum_cols[:, blk:blk + 1],
                )
                sq_t = epool.tile([128, 128], F32, tag="sq")
                nc.scalar.activation(
                    out=sq_t[:], in_=ysl,
                    func=mybir.ActivationFunctionType.Square,
                    accum_out=sumsq_cols[:, blk:blk + 1],
                )

            # ---- global BN statistics (tiny all-reduce) ----
            stats2 = cpool.tile([128, 2], F32)
            nc.vector.tensor_reduce(stats2[:, 0:1], sum_cols[:],
                                    axis=mybir.AxisListType.X,
                                    op=mybir.AluOpType.add)
            nc.vector.tensor_reduce(stats2[:, 1:2], sumsq_cols[:],
                                    axis=mybir.AxisListType.X,
                                    op=mybir.AluOpType.add)
            cc_in = dpool.tile([128, 2], F32)
            cc_out = dpool.tile([128, 2], F32, addr_space="Shared")
            nc.sync.dma_start(out=cc_in[:], in_=stats2[:])
            nc.gpsimd.collective_compute(
                "AllReduce",
                mybir.AluOpType.add,
                replica_groups=[list(range(N_CORES))],
                ins=[cc_in.opt()],
                outs=[cc_out.opt()],
            )
            gst = cpool.tile([128, 2], F32)
            nc.sync.dma_start(out=gst[:], in_=cc_out[:])

            inv_n = 1.0 / float(N_NODES)
            mean = cpool.tile([128, 1], F32)
            nc.vector.tensor_scalar(out=mean[:], in0=gst[:, 0:1],
                                    scalar1=inv_n, scalar2=None,
                                    op0=mybir.AluOpType.mult)
            ex2 = cpool.tile([128, 1], F32)
            nc.vector.tensor_scalar(out=ex2[:], in0=gst[:, 1:2],
                                    scalar1=inv_n, scalar2=None,
                                    op0=mybir.AluOpType.mult)
            mean2 = cpool.tile([128, 1], F32)
            nc.vector.tensor_tensor(out=mean2[:], in0=mean[:], in1=mean[:],
                                    op=mybir.AluOpType.mult)
            var = cpool.tile([128, 1], F32)
            nc.vector.tensor_tensor(out=var[:], in0=ex2[:], in1=mean2[:],
                                    op=mybir.AluOpType.subtract)
            eps_t = cpool.tile([128, 1], F32)
            nc.vector.memset(eps_t[:], float(BN_EPS))
            sdv = cpool.tile([128, 1], F32)
            nc.scalar.activation(out=sdv[:], in_=var[:],
                                 func=mybir.ActivationFunctionType.Sqrt,
                                 bias=eps_t[:])
            inv_std = cpool.tile([128, 1], F32)
            nc.vector.reciprocal(inv_std[:], sdv[:])
            a_col = cpool.tile([128, 1], F32)
            nc.vector.tensor_tensor(out=a_col[:],
                                    in0=meta_sb[:, 2 * C + 128:2 * C + 129],
                                    in1=inv_std[:], op=mybir.AluOpType.mult)
            ma = cpool.tile([128, 1], F32)
            nc.vector.tensor_tensor(out=ma[:], in0=mean[:], in1=a_col[:],
                                    op=mybir.AluOpType.mult)
            c_col = cpool.tile([128, 1], F32)
            nc.vector.tensor_tensor(out=c_col[:],
                                    in0=meta_sb[:, 2 * C + 129:2 * C + 130],
                                    in1=ma[:], op=mybir.AluOpType.subtract)

            # ---- apply BN + ReLU, transpose back, write out ----
            for blk in range(BLOCKS):
                yn = opool.tile([128, 128], F32, tag="yn")
                nc.scalar.activation(
                    out=yn[:], in_=y_all[:, blk * 128:(blk + 1) * 128],
                    func=mybir.ActivationFunctionType.Relu,
                    bias=c_col[:], scale=a_col[:],
                )
                t_ps = psT.tile([128, 128], F32, tag="t", space="PSUM")
                nc.tensor.transpose(t_ps[:], yn[:], ident[:])
                osb = opool.tile([128, 128], F32, tag="osb")
                nc.vector.tensor_copy(osb[:], t_ps[:])
                nc.sync.dma_start(out=out_d[blk * 128:(blk + 1) * 128, :],
                                  in_=osb[:])
    nc.finalize()
    return nc


def _preprocess(edge_index):
    """Bucket edges by target into per-core chunk arrays.

    Self-loops are handled separately (diag term), but the degree counts
    include them, matching the reference.
    """
    row = np.asarray(edge_index[0], dtype=np.int64)
    col = np.asarray(edge_index[1], dtype=np.int64)

    deg = (np.bincount(col, minlength=N_NODES) + 1).astype(np.float32)
    dinv = 1.0 / np.sqrt(np.maximum(deg, 1.0))
    norm = (dinv[row] * dinv[col]).astype(np.float32)
    dinv2 = (dinv * dinv).astype(np.float32)

    order = np.argsort(col, kind="stable")
    row_s = row[order]
    col_s = col[order]
    norm_s = norm[order]

    n_blocks_tot = N_CORES * BLOCKS
    b = (col_s >> 7).astype(np.int64)          # global 128-target block id
    cnt = np.bincount(b, minlength=n_blocks_tot)
    K = int(np.ceil(cnt.max() / 128.0))
    K = max(K, 1)
    C = BLOCKS * K

    starts = np.concatenate([[0], np.cumsum(cnt)])[:-1]
    r = np.arange(len(col_s)) - starts[b]       # rank within block
    chunk_in_blk = r // 128
    lane = r % 128
    core = b // BLOCKS
    blk_local = b % BLOCKS
    chunk_col = blk_local * K + chunk_in_blk

    srcs = np.zeros((N_CORES, C, 128), dtype=np.int32)
    tloc = np.zeros((N_CORES, C, 128), dtype=np.float32)
    nrm = np.zeros((N_CORES, C, 128), dtype=np.float32)
    srcs[core, chunk_col, lane] = row_s.astype(np.int32)
    tloc[core, chunk_col, lane] = (col_s & 127).astype(np.float32)
    nrm[core, chunk_col, lane] = norm_s

    # device wants [128 partitions, C chunks]
    srcs = np.ascontiguousarray(srcs.transpose(0, 2, 1))
    tloc = np.ascontiguousarray(tloc.transpose(0, 2, 1))
    nrm = np.ascontiguousarray(nrm.transpose(0, 2, 1))

    # dinv^2 per (core, block-lane): [8, 128, BLOCKS]
    d2 = np.zeros(N_CORES * NSH, dtype=np.float32)
    d2[:N_NODES] = dinv2
    d2 = d2.reshape(N_CORES, BLOCKS, 128).transpose(0, 2, 1)
    d2 = np.ascontiguousarray(d2)
    return K, srcs, tloc, nrm, d2


def _build_meta(K, tloc, nrm, d2, gamma, beta):
    """Pack tloc | norm | iota | gamma | beta | dinv2 into one meta array."""
    C = BLOCKS * K
    meta = np.zeros((N_CORES, 128, 2 * C + 130 + BLOCKS), dtype=np.float32)
    iota = np.tile(np.arange(128, dtype=np.float32)[None, :], (128, 1))
    for k in range(N_CORES):
        meta[k, :, 0:C] = tloc[k]
        meta[k, :, C:2 * C] = nrm[k]
        meta[k, :, 2 * C:2 * C + 128] = iota
        meta[k, :, 2 * C + 128] = gamma
        meta[k, :, 2 * C + 129] = beta
        meta[k, :, 2 * C + 130:] = d2[k]
    return meta


def kernel(x, edge_index, W, b, gamma, beta, _trace=False):
    global LAST_RESULTS
    x = np.ascontiguousarray(np.asarray(x, dtype=np.float32))
    W = np.ascontiguousarray(np.asarray(W, dtype=np.float32))
    gamma = np.ascontiguousarray(np.asarray(gamma, dtype=np.float32))
    beta = np.ascontiguousarray(np.asarray(beta, dtype=np.float32))

    K, srcs, tloc, nrm, d2 = _preprocess(np.asarray(edge_index))
    meta = _build_meta(K, tloc, nrm, d2, gamma, beta)

    xsh = np.zeros((N_CORES * NSH, HIDDEN), dtype=np.float32)
    xsh[:N_NODES] = x
    xsh = xsh.reshape(N_CORES, NSH, HIDDEN)

    if K not in _compiled:
        _compiled[K] = _build_program(K)
    nc = _compiled[K]

    in_maps = []
    for k in range(N_CORES):
        in_maps.append({
            "x": x,
            "xsh": np.ascontiguousarray(xsh[k]),
            "w_in": W,
            "meta": meta[k],
            "srcs": srcs[k],
        })
    global _K_last, _in_maps_last
    _K_last = K
    _in_maps_last = in_maps
    res = run_bass_kernel_spmd(nc, in_maps, core_ids=list(range(N_CORES)),
                               trace=_trace)
    LAST_RESULTS = res
    outs = [res.results[k]["out"] for k in range(N_CORES)]
    full = np.concatenate(outs, axis=0)[:N_NODES]
    return np.ascontiguousarray(full.astype(np.float32))
